# revision 13
# baseline (speedup 1.0000x reference)
"""Trainium2 Bass kernel for a dense transformer block (pre-LN, RoPE, causal
attention, GELU MLP) on 8 NeuronCores — fp8 (e4m3) DoubleRow edition.

Sharding: LN1 token-sharded (512 rows/core); the core transposes+casts its own
rows to fp8 [d, tok] layout and an AllGather distributes the transposed
activations (1MB/rank). QKV is head-sharded (2 heads/core, all tokens), with
RoPE applied to q/k in bf16 (per-head scores/softmax in bf16, AV in fp8
DoubleRow). Attention outputs are normalized on-core and exchanged with a
per-batch AllToAll (0.5MB/rank) so each core computes out_proj + residual +
LN2 + the whole MLP for only its own 512 token rows. All large GEMMs (QKV,
out_proj, FF1, FF2) run as fp8e4m3 DoubleRow matmuls (2x128 contraction per
pass) with weights pre-scaled by 2^12 (2^13 for FF2) on the host; descales are
folded into the RoPE trig tables / epilogue affine ops. fp32 PSUM throughout.
Softmax is computed without max-subtraction (scores provably < ~3) with the
causal mask applied multiplicatively post-exp. gpsimd carries only the
collectives.
"""

import math
from contextlib import ExitStack

import numpy as np
import ml_dtypes

import concourse.bass as bass
import concourse.bacc as bacc
import concourse.tile as tile
import concourse.mybir as mybir

BF16 = mybir.dt.bfloat16
F32 = mybir.dt.float32
F8 = mybir.dt.float8e4
AF = mybir.ActivationFunctionType
ALU = mybir.AluOpType
DR = mybir.MatmulPerfMode.DoubleRow

N_CORES = 8
B, S, D = 2, 2048, 2048
H, DH, DFF = 16, 128, 8192
NT = B * S
P = 128
ROWS = NT // N_CORES   # 512 own rows per core (256 per batch)
EPS = 1e-5
ATT_SCALE = 1.0 / math.sqrt(DH)
SW = 4096.0            # weight scale for wqkv/wout/wff1
SW2 = 8192.0           # weight scale for wff2
RG = [list(range(N_CORES))]

_EXEC_CACHE = {}


def _ln_apply(nc, pool, eps_t, x_t, out_t):
    """LayerNorm over free dim of x_t [128, 2048] -> out_t (gain/bias folded
    into downstream weights on host)."""
    st = pool.tile([P, 4, 6], F32, tag="bnst", bufs=4, name="bnst")
    for sg in range(4):
        nc.vector.bn_stats(out=st[:, sg, :], in_=x_t[:, sg * 512:(sg + 1) * 512])
    mv = pool.tile([P, 2], F32, tag="bnmv", bufs=4, name="bnmv")
    nc.vector.bn_aggr(out=mv, in_=st)
    rstd = pool.tile([P, 1], F32, tag="rstd", bufs=4, name="rstd")
    nc.scalar.activation(rstd, mv[:, 1:2], AF.Sqrt, bias=eps_t, scale=1.0)
    nc.vector.reciprocal(out=rstd, in_=rstd)
    nc.vector.tensor_scalar(out=out_t, in0=x_t, scalar1=mv[:, 0:1],
                            scalar2=rstd, op0=ALU.subtract, op1=ALU.mult)


def _emit_p1(nc, tc, gl, consts, dram_io, dbufs, it, no_cc):
    """LN1 over own 512 rows -> transpose -> fp8 pack -> AllGather."""
    x_bf_d = dram_io["x_bf"]
    xn_d, xnT_sh, xnT_all = dbufs["xn_d"], dbufs["xnT_sh"], dbufs["xnT_all"]
    with ExitStack() as c:
        pool = c.enter_context(tc.tile_pool(name=f"p1_{it}", bufs=1))
        for tt in range(4):
            x_t = pool.tile([P, D], BF16, tag="x_t", bufs=3, name="x_t")
            nc.sync.dma_start(x_t, x_bf_d[tt * P:(tt + 1) * P, :])
            xn_t = pool.tile([P, D], BF16, tag="xn_t", bufs=2, name="xn_t")
            _ln_apply(nc, pool, consts["eps"], x_t, xn_t)
            nc.scalar.dma_start(xn_d[tt * P:(tt + 1) * P, :], xn_t)
        pack8 = pool.tile([P, 16, ROWS], F8, tag="pack8", bufs=1, name="pack8")
        for cc in range(16):
            tr = pool.tile([P, ROWS], BF16, tag="tr", bufs=3, name="tr")
            nc.scalar.dma_start_transpose(tr, xn_d[:, cc * P:(cc + 1) * P])
            nc.vector.tensor_copy(out=pack8[:, cc, :], in_=tr)
        nc.sync.dma_start(
            xnT_sh[:, :].rearrange("(c p) t -> p c t", p=P), pack8)
    if no_cc:
        nc.sync.dma_start(xnT_all[0:D, :], xnT_sh[:, :])
    else:
        nc.gpsimd.collective_compute(
            "AllGather", ALU.bypass, replica_groups=RG,
            ins=[xnT_sh.opt()], outs=[xnT_all.opt()])


def _emit_p234(nc, tc, gl, consts, dram_io, dbufs, it, no_cc):
    """QKV + RoPE + attention + A2A + out_proj + LN2 + xn2 transpose/pack."""
    x_bf_d = dram_io["x_bf"]
    wout_d = dram_io["wout"]
    xnT_all = dbufs["xnT_all"]
    aT_sh, aT_all = dbufs["aT_sh"], dbufs["aT_all"]
    x2_d = dbufs["x2_d"]
    ident, maskT, rotT = consts["ident"], consts["maskT"], consts["rotT"]
    ones1, ones3 = consts["ones1"], consts["ones3"]
    bqkv, cosT, sinT = consts["bqkv"], consts["cosT"], consts["sinT"]
    bout_bc = consts["bout_bc"]
    wqkv_sb = gl["wqkv_sb"]

    with ExitStack() as cA:
        pAt = cA.enter_context(tc.tile_pool(name=f"pAt_{it}", bufs=1))
        q_sb = [pAt.tile([P, NT], BF16, tag="qk", bufs=4, name=f"q{h}")
                for h in range(2)]
        k_sb = [pAt.tile([P, NT], BF16, tag="qk", bufs=4, name=f"k{h}")
                for h in range(2)]
        vT = [[pAt.tile([P, 4, P], F8, tag="vT", bufs=16, name=f"vT{h}_{g}")
               for g in range(8)] for h in range(2)]

        # ---- QKV + RoPE + v-transpose ----
        with ExitStack() as cQ:
            pQ = cQ.enter_context(tc.tile_pool(name=f"pQ_{it}", bufs=1))
            psQ = cQ.enter_context(
                tc.tile_pool(name=f"psQ_{it}", bufs=1, space="PSUM"))
            for b_ in range(B):
                for tg in range(4):
                    g = b_ * 4 + tg
                    xnq = gl["pool"].tile([P, 16, 512], F8, tag="xnq", bufs=3,
                                          name="xnq")
                    for jj in range(2):
                        r = 2 * tg + jj
                        src = xnT_all[r * D:(r + 1) * D,
                                      b_ * 256:(b_ + 1) * 256]
                        nc.sync.dma_start(
                            xnq[:, :, jj * 256:(jj + 1) * 256],
                            src.rearrange("(c p) t -> p c t", p=P))
                    for cb in range(6):
                        ps = psQ.tile([P, 512], F32, tag="mm", bufs=4,
                                      name="qkvps")
                        for cc in range(8):
                            nc.tensor.matmul(
                                ps,
                                lhsT=wqkv_sb[:, 2 * cc:2 * cc + 2,
                                             cb * P:(cb + 1) * P],
                                rhs=xnq[:, 2 * cc:2 * cc + 2, :],
                                start=(cc == 0), stop=(cc == 7), perf_mode=DR)
                        seg = pQ.tile([P, 512], BF16, tag="seg", bufs=3,
                                      name="seg")
                        if cb < 4:
                            nc.scalar.activation(
                                seg, ps, AF.Identity,
                                bias=bqkv[:, cb:cb + 1], scale=1.0)
                        else:
                            nc.vector.tensor_scalar_add(
                                out=seg, in0=ps, scalar1=bqkv[:, cb:cb + 1])
                        if cb < 4:
                            h = cb % 2
                            rps = psQ.tile([P, 512], F32, tag="rpstp", bufs=2,
                                           name="ropeps")
                            nc.tensor.matmul(rps, lhsT=rotT, rhs=seg,
                                             start=True, stop=True)
                            t1 = pQ.tile([P, 512], BF16, tag="ropet1", bufs=2,
                                         name="ropet1")
                            nc.vector.tensor_mul(
                                t1, seg, cosT[:, tg * 512:(tg + 1) * 512])
                            t2 = pQ.tile([P, 512], BF16, tag="ropet2", bufs=2,
                                         name="ropet2")
                            nc.vector.tensor_mul(
                                t2, rps, sinT[:, tg * 512:(tg + 1) * 512])
                            dest = (q_sb[h] if cb < 2 else k_sb[h])
                            nc.vector.tensor_add(
                                dest[:, g * 512:(g + 1) * 512], t1, t2)
                        else:
                            h = cb - 4
                            tp = psQ.tile([P, 512], BF16, tag="rpstp", bufs=2,
                                          name="tpv")
                            for sub in range(4):
                                nc.tensor.transpose(
                                    tp[:, sub * P:(sub + 1) * P],
                                    seg[:, sub * P:(sub + 1) * P], ident)
                            # descale v (seg carries 2^12) while casting to fp8
                            nc.vector.tensor_scalar_mul(
                                out=vT[h][g], in0=tp, scalar1=1.0 / SW)

        # ---- attention per batch + A2A ----
        with ExitStack() as cS:
            psS = cS.enter_context(
                tc.tile_pool(name=f"psS_{it}", bufs=1, space="PSUM"))
            for b_ in range(B):
                for qg in range(4):
                    nkb = 4 * qg + 4
                    for h in range(2):
                        av_ps = psS.tile([P, 512], F32, tag="av", bufs=3,
                                         name="avps")
                        dn_ps = psS.tile([1, 512], F32, tag="dnrec", bufs=2,
                                         name="dnps")
                        eT_list = {}

                        def emit_pair(pr, h=h, b_=b_, qg=qg):
                            eT = pAt.tile([P, 2, 512], F8, tag="eT", bufs=4,
                                          name="eT")
                            for jj in range(2):
                                kb = 2 * pr + jj
                                sub0 = max(0, kb - 4 * qg)
                                w_ = 512 - sub0 * P
                                sc_ps = psS.tile([P, 512], F32, tag="sc",
                                                 bufs=3, name="scps")
                                nc.tensor.matmul(
                                    sc_ps[:, 0:w_],
                                    lhsT=k_sb[h][:, b_ * S + kb * P:
                                                 b_ * S + (kb + 1) * P],
                                    rhs=q_sb[h][:, b_ * S + qg * 512 + sub0 * P:
                                                b_ * S + (qg + 1) * 512],
                                    start=True, stop=True)
                                if sub0 > 0:
                                    nc.vector.memset(eT[:, jj, 0:sub0 * P], 0.0)
                                nc.scalar.activation(
                                    eT[:, jj, sub0 * P:], sc_ps[:, 0:w_],
                                    AF.Exp, scale=ATT_SCALE)
                                if kb >= 4 * qg:
                                    ssl = eT[:, jj, (kb - 4 * qg) * P:
                                             (kb - 4 * qg + 1) * P]
                                    nc.vector.tensor_mul(ssl, ssl, maskT)
                            eT_list[pr] = eT

                        npr = nkb // 2
                        emit_pair(0)
                        for pr in range(npr):
                            if pr + 1 < npr:
                                emit_pair(pr + 1)
                            eT = eT_list.pop(pr)
                            g = b_ * 4 + pr // 2
                            c0 = 2 * (pr % 2)
                            nc.tensor.matmul(
                                dn_ps, lhsT=ones3, rhs=eT,
                                start=(pr == 0), stop=(pr == npr - 1),
                                perf_mode=DR)
                            nc.tensor.matmul(
                                av_ps, lhsT=vT[h][g][:, c0:c0 + 2, :], rhs=eT,
                                start=(pr == 0), stop=(pr == npr - 1),
                                perf_mode=DR)
                        # normalize: rec broadcast along partitions via matmul
                        rec1 = pAt.tile([1, 512], BF16, tag="rec1", bufs=2,
                                        name="rec1")
                        with nc.allow_low_precision(reason="softmax denom"):
                            nc.vector.reciprocal(out=rec1, in_=dn_ps)
                        rec_ps = psS.tile([P, 512], F32, tag="dnrec", bufs=2,
                                          name="recps")
                        nc.tensor.matmul(rec_ps, lhsT=ones1, rhs=rec1,
                                         start=True, stop=True)
                        rec_sb = pAt.tile([P, 512], BF16, tag="rec_sb",
                                          bufs=3, name="rec_sb")
                        nc.vector.tensor_copy(out=rec_sb, in_=rec_ps)
                        aT8 = pAt.tile([P, 512], F8, tag="aT8", bufs=4,
                                       name="aT8")
                        nc.vector.tensor_mul(aT8, av_ps, rec_sb)
                        for jj in range(2):
                            r = 2 * qg + jj
                            nc.scalar.dma_start(
                                aT_sh[b_][r * 256 + h * P:
                                          r * 256 + (h + 1) * P, :],
                                aT8[:, jj * 256:(jj + 1) * 256])
                if no_cc:
                    nc.sync.dma_start(aT_all[b_][0:256, :],
                                      aT_sh[b_][0:256, :])
                else:
                    nc.gpsimd.collective_compute(
                        "AllToAll", ALU.bypass, replica_groups=RG,
                        ins=[aT_sh[b_].opt()],
                        outs=[aT_all[b_].opt()])

        # ---- out_proj + residual + LN2 (own 512 rows) ----
        xn2q = gl["pool"].tile([P, 16, ROWS], F8, tag="xn2q", bufs=2,
                               name="xn2q")
        with ExitStack() as cO:
            pO = cO.enter_context(tc.tile_pool(name=f"pO_{it}", bufs=1))
            psO = cO.enter_context(
                tc.tile_pool(name=f"psO_{it}", bufs=1, space="PSUM"))
            wo_tiles = []
            for b_ in range(B):
                aTl_all = pO.tile([P, 16, 256], F8, tag="aTl", bufs=2,
                                  name="aTl")
                nc.sync.dma_start(
                    aTl_all, aT_all[b_][:, :]
                    .rearrange("(c p) t -> p c t", p=P))
                xrs, x2s = [], []
                for tb in range(2):
                    ib = b_ * 2 + tb
                    xr = pO.tile([P, D], BF16, tag="xr", bufs=4, name="xr")
                    nc.sync.dma_start(xr, x_bf_d[ib * P:(ib + 1) * P, :])
                    nc.vector.tensor_add(xr, xr, bout_bc)
                    xrs.append(xr)
                    x2s.append(pO.tile([P, D], BF16, tag="x2", bufs=4,
                                       name="x2"))
                for dg in range(4):
                    if b_ == 0:
                        wo = pO.tile([P, 16, 512], F8, tag="wo", bufs=4,
                                     name="wo")
                        nc.sync.dma_start(
                            wo,
                            wout_d[:, :].rearrange("p (c n) -> p c n", c=16)
                            [:, :, dg * 512:(dg + 1) * 512])
                        wo_tiles.append(wo)
                    else:
                        wo = wo_tiles[dg]
                    for tb in range(2):
                        op_ps = psO.tile([P, 512], F32, tag="op", bufs=3,
                                         name="opps")
                        for r in range(8):
                            nc.tensor.matmul(
                                op_ps,
                                lhsT=aTl_all[:, 2 * r:2 * r + 2,
                                             tb * P:(tb + 1) * P],
                                rhs=wo[:, 2 * r:2 * r + 2, :],
                                start=(r == 0), stop=(r == 7), perf_mode=DR)
                        nc.vector.scalar_tensor_tensor(
                            out=x2s[tb][:, dg * 512:(dg + 1) * 512],
                            in0=op_ps, scalar=1.0 / SW,
                            in1=xrs[tb][:, dg * 512:(dg + 1) * 512],
                            op0=ALU.mult, op1=ALU.add)
                for tb in range(2):
                    ib = b_ * 2 + tb
                    nc.scalar.dma_start(x2_d[ib * P:(ib + 1) * P, :], x2s[tb])
                    xn2 = pO.tile([P, D], BF16, tag="xn2", bufs=2, name="xn2")
                    _ln_apply(nc, pO, consts["eps"], x2s[tb], xn2)
                    # transpose xn2 on the PE straight into the fp8 pack (no
                    # DRAM bounce); 4 chunks share one bf16 psum bank so the
                    # fp8 cast runs as one wide DVE op per bank
                    for c4 in range(4):
                        tp2 = psO.tile([P, 4, P], BF16, tag="tp2", bufs=2,
                                       name="tp2")
                        for sub in range(4):
                            cc = c4 * 4 + sub
                            nc.tensor.transpose(
                                tp2[:, sub, :], xn2[:, cc * P:(cc + 1) * P],
                                ident)
                        nc.vector.tensor_copy(
                            out=xn2q[:, c4 * 4:(c4 + 1) * 4,
                                     ib * P:(ib + 1) * P], in_=tp2)
    return xn2q


def _emit_p5(nc, tc, gl, consts, dram_io, dbufs, it, xn2q):
    """MLP over own 512 rows: FF1+GELU (fp8 DoubleRow) -> FF2 + residual."""
    wff1_d, wff2_d, out_d = (dram_io["wff1"], dram_io["wff2"], dram_io["out"])
    x2_d = dbufs["x2_d"]
    bff1, bff2_bc = consts["bff1"], consts["bff2_bc"]

    with ExitStack() as c:
        pM = c.enter_context(tc.tile_pool(name=f"pM_{it}", bufs=1))
        psM = c.enter_context(
            tc.tile_pool(name=f"psM_{it}", bufs=1, space="PSUM"))

        hT = pM.tile([P, 64, 512], F8, tag="hT", bufs=1, name="hT")
        for fb in range(64):
            w1 = pM.tile([P, 16, P], F8, tag="w1", bufs=6, name="w1")
            nc.sync.dma_start(
                w1, wff1_d[:, fb * 2048:(fb + 1) * 2048]
                .rearrange("p (c n) -> p c n", c=16))
            ps = psM.tile([P, 512], F32, tag="ff1", bufs=3, name="ff1ps")
            for cc in range(8):
                nc.tensor.matmul(
                    ps, lhsT=w1[:, 2 * cc:2 * cc + 2, :],
                    rhs=xn2q[:, 2 * cc:2 * cc + 2, :],
                    start=(cc == 0), stop=(cc == 7), perf_mode=DR)
            nc.scalar.activation(hT[:, fb, :], ps, AF.Gelu,
                                 bias=bff1[:, fb:fb + 1], scale=1.0 / SW)

        x2pb = []
        for ib in range(4):
            xt = pM.tile([P, D], BF16, tag="x2pb", bufs=4, name=f"x2pb{ib}")
            nc.sync.dma_start(xt, x2_d[ib * P:(ib + 1) * P, :])
            nc.vector.tensor_add(xt, xt, bff2_bc)
            x2pb.append(xt)

        for dq in range(4):
            psums = [psM.tile([P, 512], F32, tag="ff2", bufs=4, name=f"f2{tb}")
                     for tb in range(4)]
            for qtr in range(4):
                w2 = pM.tile([P, 16, 512], F8, tag="w2", bufs=5, name="w2")
                nc.sync.dma_start(
                    w2, wff2_d[:, :].rearrange("p (c n) -> p c n", c=64)
                    [:, qtr * 16:(qtr + 1) * 16, dq * 512:(dq + 1) * 512])
                for cc in range(8):
                    for tb in range(4):
                        nc.tensor.matmul(
                            psums[tb],
                            lhsT=hT[:, qtr * 16 + 2 * cc:
                                    qtr * 16 + 2 * cc + 2,
                                    tb * P:(tb + 1) * P],
                            rhs=w2[:, 2 * cc:2 * cc + 2, :],
                            start=(qtr == 0 and cc == 0),
                            stop=(qtr == 3 and cc == 7), perf_mode=DR)
            for tb in range(4):
                o1 = pM.tile([P, 512], F32, tag="o1", bufs=4, name="o1")
                nc.vector.scalar_tensor_tensor(
                    out=o1, in0=psums[tb], scalar=1.0 / SW2,
                    in1=x2pb[tb][:, dq * 512:(dq + 1) * 512],
                    op0=ALU.mult, op1=ALU.add)
                nc.scalar.dma_start(
                    out_d[tb * P:(tb + 1) * P, dq * 512:(dq + 1) * 512], o1)


def build_program(loop=1, no_cc=False):
    nc = bacc.Bacc("TRN2", target_bir_lowering=False, debug=False,
                   num_devices=N_CORES)

    dram_io = {
        "x_bf": nc.dram_tensor("x_bf", [ROWS, D], BF16, kind="ExternalInput"),
        "wqkv": nc.dram_tensor("wqkv", [P, 16 * 768], F8, kind="ExternalInput"),
        "bqkv": nc.dram_tensor("bqkv", [P, 6], F32, kind="ExternalInput"),
        "cosT": nc.dram_tensor("cosT", [P, S], BF16, kind="ExternalInput"),
        "sinT": nc.dram_tensor("sinT", [P, S], BF16, kind="ExternalInput"),
        "rotT": nc.dram_tensor("rotT", [P, P], BF16, kind="ExternalInput"),
        "maskT": nc.dram_tensor("maskT", [P, P], BF16, kind="ExternalInput"),
        "ident": nc.dram_tensor("ident", [P, P], BF16, kind="ExternalInput"),
        "wout": nc.dram_tensor("wout", [P, 16 * 2048], F8, kind="ExternalInput"),
        "bout": nc.dram_tensor("bout", [D], BF16, kind="ExternalInput"),
        "wff1": nc.dram_tensor("wff1", [P, 64 * 2048], F8, kind="ExternalInput"),
        "bff1": nc.dram_tensor("bff1", [P, 64], F32, kind="ExternalInput"),
        "wff2": nc.dram_tensor("wff2", [P, 64 * 2048], F8, kind="ExternalInput"),
        "bff2": nc.dram_tensor("bff2", [D], BF16, kind="ExternalInput"),
        "out": nc.dram_tensor("out", [ROWS, D], F32, kind="ExternalOutput"),
    }

    with tile.TileContext(nc) as tc:
        with ExitStack() as ctx:
            cpool = ctx.enter_context(tc.tile_pool(name="consts", bufs=1))
            gpool = ctx.enter_context(tc.tile_pool(name="glob", bufs=1))
            dram = ctx.enter_context(
                tc.tile_pool(name="dram", bufs=1, space="DRAM"))

            def cload(name, shape, dt):
                t = cpool.tile(shape, dt, name=name + "_sb")
                nc.sync.dma_start(t, dram_io[name][:, :])
                return t

            ident = cload("ident", [P, P], BF16)
            maskT = cload("maskT", [P, P], BF16)
            rotT = cload("rotT", [P, P], BF16)
            bqkv = cload("bqkv", [P, 6], F32)
            bff1 = cload("bff1", [P, 64], F32)
            cosT = cload("cosT", [P, S], BF16)
            sinT = cload("sinT", [P, S], BF16)
            ones1 = cpool.tile([1, P], BF16, name="ones1")
            nc.vector.memset(ones1, 1.0)
            ones3_full = cpool.tile([P, 2, 16], F8, name="ones3")
            nc.vector.memset(ones3_full, 1.0)
            ones3 = ones3_full[:, :, 0:1]  # j-step 16B for DoubleRow
            eps_t = cpool.tile([P, 1], F32, name="eps_sb")
            nc.vector.memset(eps_t, EPS)
            bout_bc = cpool.tile([P, D], BF16, name="bout_bc")
            nc.gpsimd.dma_start(
                out=bout_bc, in_=dram_io["bout"].ap()[None, :]
                .to_broadcast((P, D)))
            bff2_bc = cpool.tile([P, D], BF16, name="bff2_bc")
            nc.gpsimd.dma_start(
                out=bff2_bc, in_=dram_io["bff2"].ap()[None, :]
                .to_broadcast((P, D)))
            consts = dict(ident=ident, maskT=maskT, rotT=rotT, bqkv=bqkv,
                          bff1=bff1, cosT=cosT, sinT=sinT, ones1=ones1,
                          ones3=ones3, eps=eps_t, bout_bc=bout_bc,
                          bff2_bc=bff2_bc)

            wqkv_sb = gpool.tile([P, 16, 768], F8, name="wqkv_sb")
            nc.sync.dma_start(
                wqkv_sb, dram_io["wqkv"][:, :]
                .rearrange("p (c n) -> p c n", c=16))
            gl = {"pool": gpool, "wqkv_sb": wqkv_sb}

            def mkbufs(it):
                return {
                    "xn_d": dram.tile([ROWS, D], BF16, name=f"xn_d{it}"),
                    "xnT_sh": dram.tile([D, ROWS], F8, name=f"xnTsh{it}"),
                    "xnT_all": dram.tile([N_CORES * D, ROWS], F8,
                                         name=f"xnTall{it}",
                                         addr_space="Shared"),
                    "aT_sh": [dram.tile([2048, 256], F8, name=f"aTsh{it}_{b}")
                              for b in range(B)],
                    "aT_all": [dram.tile([2048, 256], F8,
                                         name=f"aTall{it}_{b}")
                               for b in range(B)],
                    "x2_d": dram.tile([ROWS, D], BF16, name=f"x2d{it}"),
                }

            dbufs = {0: mkbufs(0)}
            _emit_p1(nc, tc, gl, consts, dram_io, dbufs[0], 0, no_cc)
            for it in range(loop):
                xn2q = _emit_p234(nc, tc, gl, consts, dram_io, dbufs[it], it,
                                  no_cc)
                if it + 1 < loop:
                    dbufs[it + 1] = mkbufs(it + 1)
                    _emit_p1(nc, tc, gl, consts, dram_io, dbufs[it + 1],
                             it + 1, no_cc)
                _emit_p5(nc, tc, gl, consts, dram_io, dbufs[it], it, xn2q)
                del dbufs[it]

    nc.compile()
    return nc


# ----------------------------------------------------------------------------
# host side
# ----------------------------------------------------------------------------

def _bf(a):
    return np.asarray(a, np.float32).astype(ml_dtypes.bfloat16)


def _f8(a, scale):
    return np.clip(np.asarray(a, np.float32) * scale, -240.0, 240.0).astype(
        ml_dtypes.float8_e4m3)


def prepare_inputs(x, cos, sin, mask,
                   ln1_g, ln1_b, w_qkv, b_qkv, w_out, b_out,
                   ln2_g, ln2_b, w_ff1, b_ff1, w_ff2, b_ff2):
    """Fold LN params into weights, shard per core, cast to device dtypes."""
    f32 = np.float32
    x2d = np.asarray(x, f32).reshape(NT, D)
    cos2 = np.asarray(cos, f32).reshape(S, DH)
    sin2 = np.asarray(sin, f32).reshape(S, DH)
    w_qkv = np.asarray(w_qkv, f32); b_qkv = np.asarray(b_qkv, f32)
    w_out = np.asarray(w_out, f32); b_out = np.asarray(b_out, f32)
    w_ff1 = np.asarray(w_ff1, f32); b_ff1 = np.asarray(b_ff1, f32)
    w_ff2 = np.asarray(w_ff2, f32); b_ff2 = np.asarray(b_ff2, f32)
    ln1_g = np.asarray(ln1_g, f32); ln1_b = np.asarray(ln1_b, f32)
    ln2_g = np.asarray(ln2_g, f32); ln2_b = np.asarray(ln2_b, f32)

    w_qkv_f = w_qkv * ln1_g[:, None]
    b_qkv_f = b_qkv + ln1_b @ w_qkv
    w_ff1_f = w_ff1 * ln2_g[:, None]
    b_ff1_f = b_ff1 + ln2_b @ w_ff1

    cosT = _bf(cos2.T / SW).copy()
    sinT = _bf(sin2.T / SW).copy()

    R = np.zeros((P, P), f32)
    for dp in range(64):
        R[dp, dp + 64] = -1.0
        R[dp + 64, dp] = 1.0
    rotT = _bf(R.T).copy()
    maskT = _bf(np.triu(np.ones((P, P), f32)))  # keep k <= q (row=k, col=q)
    ident = _bf(np.eye(P, dtype=f32))

    wff1_r = np.ascontiguousarray(
        _f8(w_ff1_f, SW).reshape(16, P, 64, P).transpose(1, 2, 0, 3)
        .reshape(P, 64 * 2048))
    bff1_r = np.ascontiguousarray(b_ff1_f.reshape(64, P).T)
    wff2_r = np.ascontiguousarray(
        _f8(w_ff2, SW2).reshape(64, P, 2048).transpose(1, 0, 2)
        .reshape(P, 64 * 2048))
    wout_r = np.ascontiguousarray(
        _f8(w_out, SW).reshape(16, P, 2048).transpose(1, 0, 2)
        .reshape(P, 16 * 2048))

    in_maps = []
    for c in range(N_CORES):
        h0 = 2 * c
        cols = np.concatenate([
            np.arange(t * D + h * DH, t * D + (h + 1) * DH)
            for t in range(3) for h in (h0, h0 + 1)])
        x_own = np.ascontiguousarray(np.concatenate([
            x2d[c * 256:(c + 1) * 256],
            x2d[S + c * 256: S + (c + 1) * 256]]))
        wq = np.ascontiguousarray(
            _f8(w_qkv_f[:, cols], SW).reshape(16, P, 768)
            .transpose(1, 0, 2).reshape(P, 16 * 768))
        in_maps.append({
            "x_bf": _bf(x_own),
            "wqkv": wq,
            "bqkv": np.ascontiguousarray(
                (b_qkv_f[cols] * SW).reshape(6, P).T),
            "cosT": cosT, "sinT": sinT, "rotT": rotT,
            "maskT": maskT, "ident": ident,
            "wout": wout_r,
            "bout": _bf(b_out),
            "wff1": wff1_r, "bff1": bff1_r,
            "wff2": wff2_r, "bff2": _bf(b_ff2),
        })
    return in_maps


class SpmdExec:
    """Compile once; run the SPMD program on 8 cores without donation so the
    call can be repeated for timing."""

    def __init__(self, nc):
        import jax
        from jax.sharding import Mesh, PartitionSpec
        from jax.experimental.shard_map import shard_map
        from concourse import bass2jax

        bass2jax.install_neuronx_cc_hook()
        self._jax = jax
        self.nc = nc
        pname = nc.partition_id_tensor.name if nc.partition_id_tensor else None
        in_names, out_names, out_avals, zeros = [], [], [], []
        for alloc in nc.m.functions[0].allocations:
            if not isinstance(alloc, mybir.MemoryLocationSet):
                continue
            name = alloc.memorylocations[0].name
            if alloc.kind == "ExternalInput":
                if name != pname:
                    in_names.append(name)
            elif alloc.kind == "ExternalOutput":
                out_names.append(name)
                shape = tuple(alloc.tensor_shape)
                dtype = mybir.dt.np(alloc.dtype)
                out_avals.append(jax.core.ShapedArray(shape, dtype))
                zeros.append(np.zeros(shape, dtype))
        self.in_names, self.out_names = in_names, out_names
        self.out_avals = out_avals
        n_params = len(in_names)
        all_names = in_names + out_names + ([pname] if pname else [])

        def _body(*args):
            ops = list(args)
            if pname:
                ops.append(bass2jax.partition_id_tensor())
            outs = bass2jax._bass_exec_p.bind(
                *ops, out_avals=tuple(out_avals), in_names=tuple(all_names),
                out_names=tuple(out_names), lowering_input_output_aliases=(),
                sim_require_finite=False, sim_require_nnan=False, nc=nc)
            return tuple(outs)

        devices = jax.devices()[:N_CORES]
        mesh = Mesh(np.asarray(devices), ("core",))
        in_specs = (PartitionSpec("core"),) * (n_params + len(out_names))
        out_specs = (PartitionSpec("core"),) * len(out_names)
        self.fn = jax.jit(
            shard_map(_body, mesh=mesh, in_specs=in_specs, out_specs=out_specs,
                      check_rep=False),
            keep_unused=True)
        self._zeros = zeros
        self._dev_args = None

    def place(self, in_maps):
        jax = self._jax
        from jax.sharding import Mesh, PartitionSpec, NamedSharding
        devices = jax.devices()[:N_CORES]
        mesh = Mesh(np.asarray(devices), ("core",))
        concat_in = [np.concatenate([np.asarray(in_maps[c][n])
                                     for c in range(N_CORES)], axis=0)
                     for n in self.in_names]
        concat_zero = [np.zeros((N_CORES * z.shape[0], *z.shape[1:]), z.dtype)
                       for z in self._zeros]
        self._dev_args = [
            jax.device_put(a, NamedSharding(mesh, PartitionSpec("core")))
            for a in concat_in + concat_zero]

    def run_raw(self):
        return self._jax.block_until_ready(self.fn(*self._dev_args))

    def run(self):
        jax = self._jax
        outs = jax.block_until_ready(self.fn(*self._dev_args))
        res = []
        for c in range(N_CORES):
            res.append({
                name: np.asarray(outs[i]).reshape(
                    N_CORES, *self.out_avals[i].shape)[c]
                for i, name in enumerate(self.out_names)})
        return res


def get_exec(loop=1, no_cc=False):
    key = (loop, no_cc)
    if key not in _EXEC_CACHE:
        nc = build_program(loop, no_cc=no_cc)
        _EXEC_CACHE[key] = SpmdExec(nc)
    return _EXEC_CACHE[key]


def assemble_output(res):
    out = np.zeros((NT, D), np.float32)
    for c in range(N_CORES):
        o = res[c]["out"]
        out[c * 256:(c + 1) * 256] = o[:256]
        out[S + c * 256: S + (c + 1) * 256] = o[256:]
    return out.reshape(B, S, D)


def kernel(**inputs):
    ex = get_exec(loop=1)
    in_maps = prepare_inputs(**inputs)
    ex.place(in_maps)
    res = ex.run()
    return assemble_output(res).astype(np.float32)


# revision 14
# speedup vs baseline: 1.0170x; 1.0170x over previous
"""Trainium2 Bass kernel for a dense transformer block (pre-LN, RoPE, causal
attention, GELU MLP) on 8 NeuronCores — fp8 (e4m3) DoubleRow edition.

Sharding: LN1 token-sharded (512 rows/core); the core transposes+casts its own
rows to fp8 [d, tok] layout and an AllGather distributes the transposed
activations (1MB/rank). QKV is head-sharded (2 heads/core, all tokens), with
RoPE applied to q/k in bf16 (per-head scores/softmax in bf16, AV in fp8
DoubleRow). Attention outputs are normalized on-core and exchanged with a
per-batch AllToAll (0.5MB/rank) so each core computes out_proj + residual +
LN2 + the whole MLP for only its own 512 token rows. All large GEMMs (QKV,
out_proj, FF1, FF2) run as fp8e4m3 DoubleRow matmuls (2x128 contraction per
pass) with weights pre-scaled by 2^12 (2^13 for FF2) on the host; descales are
folded into the RoPE trig tables / epilogue affine ops. fp32 PSUM throughout.
Softmax is computed without max-subtraction (scores provably < ~3) with the
causal mask applied multiplicatively post-exp. gpsimd carries only the
collectives.
"""

import math
from contextlib import ExitStack

import numpy as np
import ml_dtypes

import concourse.bass as bass
import concourse.bacc as bacc
import concourse.tile as tile
import concourse.mybir as mybir

BF16 = mybir.dt.bfloat16
F32 = mybir.dt.float32
F8 = mybir.dt.float8e4
AF = mybir.ActivationFunctionType
ALU = mybir.AluOpType
DR = mybir.MatmulPerfMode.DoubleRow

N_CORES = 8
B, S, D = 2, 2048, 2048
H, DH, DFF = 16, 128, 8192
NT = B * S
P = 128
ROWS = NT // N_CORES   # 512 own rows per core (256 per batch)
EPS = 1e-5
ATT_SCALE = 1.0 / math.sqrt(DH)
SW = 4096.0            # weight scale for wqkv/wout/wff1
SW2 = 8192.0           # weight scale for wff2
RG = [list(range(N_CORES))]

_EXEC_CACHE = {}


def _ln_apply(nc, pool, eps_t, x_t, out_t):
    """LayerNorm over free dim of x_t [128, 2048] -> out_t (gain/bias folded
    into downstream weights on host)."""
    st = pool.tile([P, 4, 6], F32, tag="bnst", bufs=4, name="bnst")
    for sg in range(4):
        nc.vector.bn_stats(out=st[:, sg, :], in_=x_t[:, sg * 512:(sg + 1) * 512])
    mv = pool.tile([P, 2], F32, tag="bnmv", bufs=4, name="bnmv")
    nc.vector.bn_aggr(out=mv, in_=st)
    rstd = pool.tile([P, 1], F32, tag="rstd", bufs=4, name="rstd")
    nc.scalar.activation(rstd, mv[:, 1:2], AF.Sqrt, bias=eps_t, scale=1.0)
    nc.vector.reciprocal(out=rstd, in_=rstd)
    nc.vector.tensor_scalar(out=out_t, in0=x_t, scalar1=mv[:, 0:1],
                            scalar2=rstd, op0=ALU.subtract, op1=ALU.mult)


def _emit_p1(nc, tc, gl, consts, dram_io, dbufs, it, no_cc):
    """LN1 over own 512 rows -> transpose -> fp8 pack -> AllGather."""
    x_bf_d = dram_io["x_bf"]
    xn_d, xnT_sh, xnT_all = dbufs["xn_d"], dbufs["xnT_sh"], dbufs["xnT_all"]
    with ExitStack() as c:
        pool = c.enter_context(tc.tile_pool(name=f"p1_{it}", bufs=1))
        for tt in range(4):
            x_t = pool.tile([P, D], BF16, tag="x_t", bufs=3, name="x_t")
            nc.sync.dma_start(x_t, x_bf_d[tt * P:(tt + 1) * P, :])
            xn_t = pool.tile([P, D], BF16, tag="xn_t", bufs=2, name="xn_t")
            _ln_apply(nc, pool, consts["eps"], x_t, xn_t)
            nc.sync.dma_start(xn_d[tt * P:(tt + 1) * P, :], xn_t)
        pack8 = pool.tile([P, 16, ROWS], F8, tag="pack8", bufs=1, name="pack8")
        for cc in range(16):
            tr = pool.tile([P, ROWS], BF16, tag="tr", bufs=3, name="tr")
            nc.sync.dma_start_transpose(tr, xn_d[:, cc * P:(cc + 1) * P])
            nc.vector.tensor_copy(out=pack8[:, cc, :], in_=tr)
        nc.sync.dma_start(
            xnT_sh[:, :].rearrange("(c p) t -> p c t", p=P), pack8)
    if no_cc:
        nc.sync.dma_start(xnT_all[0:D, :], xnT_sh[:, :])
    else:
        nc.gpsimd.collective_compute(
            "AllGather", ALU.bypass, replica_groups=RG,
            ins=[xnT_sh.opt()], outs=[xnT_all.opt()])


def _emit_p234(nc, tc, gl, consts, dram_io, dbufs, it, no_cc):
    """QKV + RoPE + attention + A2A + out_proj + LN2 + xn2 transpose/pack."""
    x_bf_d = dram_io["x_bf"]
    wout_d = dram_io["wout"]
    xnT_all = dbufs["xnT_all"]
    aT_sh, aT_all = dbufs["aT_sh"], dbufs["aT_all"]
    x2_d = dbufs["x2_d"]
    ident, maskT, rotT = consts["ident"], consts["maskT"], consts["rotT"]
    ones1, ones3 = consts["ones1"], consts["ones3"]
    bqkv, cosT, sinT = consts["bqkv"], consts["cosT"], consts["sinT"]
    bout_bc = consts["bout_bc"]
    wqkv_sb = gl["wqkv_sb"]

    with ExitStack() as cA:
        pAt = cA.enter_context(tc.tile_pool(name=f"pAt_{it}", bufs=1))
        q_sb = [pAt.tile([P, NT], BF16, tag="qk", bufs=4, name=f"q{h}")
                for h in range(2)]
        k_sb = [pAt.tile([P, NT], BF16, tag="qk", bufs=4, name=f"k{h}")
                for h in range(2)]
        vT = [[pAt.tile([P, 4, P], F8, tag="vT", bufs=16, name=f"vT{h}_{g}")
               for g in range(8)] for h in range(2)]

        # ---- QKV + RoPE + v-transpose ----
        with ExitStack() as cQ:
            pQ = cQ.enter_context(tc.tile_pool(name=f"pQ_{it}", bufs=1))
            psQ = cQ.enter_context(
                tc.tile_pool(name=f"psQ_{it}", bufs=1, space="PSUM"))
            for b_ in range(B):
                for tg in range(4):
                    g = b_ * 4 + tg
                    xnq = gl["pool"].tile([P, 16, 512], F8, tag="xnq", bufs=3,
                                          name="xnq")
                    for jj in range(2):
                        r = 2 * tg + jj
                        src = xnT_all[r * D:(r + 1) * D,
                                      b_ * 256:(b_ + 1) * 256]
                        nc.sync.dma_start(
                            xnq[:, :, jj * 256:(jj + 1) * 256],
                            src.rearrange("(c p) t -> p c t", p=P))
                    for cb in range(6):
                        ps = psQ.tile([P, 512], F32, tag="mm", bufs=4,
                                      name="qkvps")
                        for cc in range(8):
                            nc.tensor.matmul(
                                ps,
                                lhsT=wqkv_sb[:, 2 * cc:2 * cc + 2,
                                             cb * P:(cb + 1) * P],
                                rhs=xnq[:, 2 * cc:2 * cc + 2, :],
                                start=(cc == 0), stop=(cc == 7), perf_mode=DR)
                        seg = pQ.tile([P, 512], BF16, tag="seg", bufs=3,
                                      name="seg")
                        if cb < 4:
                            nc.scalar.activation(
                                seg, ps, AF.Identity,
                                bias=bqkv[:, cb:cb + 1], scale=1.0)
                        else:
                            nc.vector.tensor_scalar_add(
                                out=seg, in0=ps, scalar1=bqkv[:, cb:cb + 1])
                        if cb < 4:
                            h = cb % 2
                            rps = psQ.tile([P, 512], F32, tag="rpstp", bufs=2,
                                           name="ropeps")
                            nc.tensor.matmul(rps, lhsT=rotT, rhs=seg,
                                             start=True, stop=True)
                            t1 = pQ.tile([P, 512], BF16, tag="ropet1", bufs=2,
                                         name="ropet1")
                            nc.vector.tensor_mul(
                                t1, seg, cosT[:, tg * 512:(tg + 1) * 512])
                            t2 = pQ.tile([P, 512], BF16, tag="ropet2", bufs=2,
                                         name="ropet2")
                            nc.vector.tensor_mul(
                                t2, rps, sinT[:, tg * 512:(tg + 1) * 512])
                            dest = (q_sb[h] if cb < 2 else k_sb[h])
                            nc.vector.tensor_add(
                                dest[:, g * 512:(g + 1) * 512], t1, t2)
                        else:
                            h = cb - 4
                            tp = psQ.tile([P, 512], BF16, tag="rpstp", bufs=2,
                                          name="tpv")
                            for sub in range(4):
                                nc.tensor.transpose(
                                    tp[:, sub * P:(sub + 1) * P],
                                    seg[:, sub * P:(sub + 1) * P], ident)
                            # descale v (seg carries 2^12) while casting to fp8
                            nc.vector.tensor_scalar_mul(
                                out=vT[h][g], in0=tp, scalar1=1.0 / SW)

        # ---- attention per batch + A2A ----
        with ExitStack() as cS:
            psS = cS.enter_context(
                tc.tile_pool(name=f"psS_{it}", bufs=1, space="PSUM"))
            for b_ in range(B):
                for qg in range(4):
                    nkb = 4 * qg + 4
                    for h in range(2):
                        av_ps = psS.tile([P, 512], F32, tag="av", bufs=3,
                                         name="avps")
                        dn_ps = psS.tile([1, 512], F32, tag="dnrec", bufs=2,
                                         name="dnps")
                        eT_list = {}

                        def emit_pair(pr, h=h, b_=b_, qg=qg):
                            eT = pAt.tile([P, 2, 512], F8, tag="eT", bufs=4,
                                          name="eT")
                            for jj in range(2):
                                kb = 2 * pr + jj
                                sub0 = max(0, kb - 4 * qg)
                                w_ = 512 - sub0 * P
                                sc_ps = psS.tile([P, 512], F32, tag="sc",
                                                 bufs=3, name="scps")
                                nc.tensor.matmul(
                                    sc_ps[:, 0:w_],
                                    lhsT=k_sb[h][:, b_ * S + kb * P:
                                                 b_ * S + (kb + 1) * P],
                                    rhs=q_sb[h][:, b_ * S + qg * 512 + sub0 * P:
                                                b_ * S + (qg + 1) * 512],
                                    start=True, stop=True)
                                if sub0 > 0:
                                    nc.vector.memset(eT[:, jj, 0:sub0 * P], 0.0)
                                nc.scalar.activation(
                                    eT[:, jj, sub0 * P:], sc_ps[:, 0:w_],
                                    AF.Exp, scale=ATT_SCALE)
                                if kb >= 4 * qg:
                                    ssl = eT[:, jj, (kb - 4 * qg) * P:
                                             (kb - 4 * qg + 1) * P]
                                    nc.vector.tensor_mul(ssl, ssl, maskT)
                            eT_list[pr] = eT

                        npr = nkb // 2
                        emit_pair(0)
                        for pr in range(npr):
                            if pr + 1 < npr:
                                emit_pair(pr + 1)
                            eT = eT_list.pop(pr)
                            g = b_ * 4 + pr // 2
                            c0 = 2 * (pr % 2)
                            nc.tensor.matmul(
                                dn_ps, lhsT=ones3, rhs=eT,
                                start=(pr == 0), stop=(pr == npr - 1),
                                perf_mode=DR)
                            nc.tensor.matmul(
                                av_ps, lhsT=vT[h][g][:, c0:c0 + 2, :], rhs=eT,
                                start=(pr == 0), stop=(pr == npr - 1),
                                perf_mode=DR)
                        # normalize: rec broadcast along partitions via matmul
                        rec1 = pAt.tile([1, 512], BF16, tag="rec1", bufs=2,
                                        name="rec1")
                        with nc.allow_low_precision(reason="softmax denom"):
                            nc.vector.reciprocal(out=rec1, in_=dn_ps)
                        rec_ps = psS.tile([P, 512], F32, tag="dnrec", bufs=2,
                                          name="recps")
                        nc.tensor.matmul(rec_ps, lhsT=ones1, rhs=rec1,
                                         start=True, stop=True)
                        rec_sb = pAt.tile([P, 512], BF16, tag="rec_sb",
                                          bufs=3, name="rec_sb")
                        nc.vector.tensor_copy(out=rec_sb, in_=rec_ps)
                        aT8 = pAt.tile([P, 512], F8, tag="aT8", bufs=4,
                                       name="aT8")
                        nc.vector.tensor_mul(aT8, av_ps, rec_sb)
                        for jj in range(2):
                            r = 2 * qg + jj
                            nc.scalar.dma_start(
                                aT_sh[b_][r * 256 + h * P:
                                          r * 256 + (h + 1) * P, :],
                                aT8[:, jj * 256:(jj + 1) * 256])
                if no_cc:
                    nc.sync.dma_start(aT_all[b_][0:256, :],
                                      aT_sh[b_][0:256, :])
                else:
                    nc.gpsimd.collective_compute(
                        "AllToAll", ALU.bypass, replica_groups=RG,
                        ins=[aT_sh[b_].opt()],
                        outs=[aT_all[b_].opt()])

        # ---- out_proj + residual + LN2 (own 512 rows) ----
        xn2q = gl["pool"].tile([P, 16, ROWS], F8, tag="xn2q", bufs=2,
                               name="xn2q")
        with ExitStack() as cO:
            pO = cO.enter_context(tc.tile_pool(name=f"pO_{it}", bufs=1))
            psO = cO.enter_context(
                tc.tile_pool(name=f"psO_{it}", bufs=1, space="PSUM"))
            wo_tiles = []
            for b_ in range(B):
                aTl_all = pO.tile([P, 16, 256], F8, tag="aTl", bufs=2,
                                  name="aTl")
                nc.sync.dma_start(
                    aTl_all, aT_all[b_][:, :]
                    .rearrange("(c p) t -> p c t", p=P))
                xrs, x2s = [], []
                for tb in range(2):
                    ib = b_ * 2 + tb
                    xr = pO.tile([P, D], BF16, tag="xr", bufs=4, name="xr")
                    nc.sync.dma_start(xr, x_bf_d[ib * P:(ib + 1) * P, :])
                    nc.vector.tensor_add(xr, xr, bout_bc)
                    xrs.append(xr)
                    x2s.append(pO.tile([P, D], BF16, tag="x2", bufs=4,
                                       name="x2"))
                for dg in range(4):
                    if b_ == 0:
                        wo = pO.tile([P, 16, 512], F8, tag="wo", bufs=4,
                                     name="wo")
                        nc.sync.dma_start(
                            wo,
                            wout_d[:, :].rearrange("p (c n) -> p c n", c=16)
                            [:, :, dg * 512:(dg + 1) * 512])
                        wo_tiles.append(wo)
                    else:
                        wo = wo_tiles[dg]
                    for tb in range(2):
                        op_ps = psO.tile([P, 512], F32, tag="op", bufs=3,
                                         name="opps")
                        for r in range(8):
                            nc.tensor.matmul(
                                op_ps,
                                lhsT=aTl_all[:, 2 * r:2 * r + 2,
                                             tb * P:(tb + 1) * P],
                                rhs=wo[:, 2 * r:2 * r + 2, :],
                                start=(r == 0), stop=(r == 7), perf_mode=DR)
                        nc.vector.scalar_tensor_tensor(
                            out=x2s[tb][:, dg * 512:(dg + 1) * 512],
                            in0=op_ps, scalar=1.0 / SW,
                            in1=xrs[tb][:, dg * 512:(dg + 1) * 512],
                            op0=ALU.mult, op1=ALU.add)
                for tb in range(2):
                    ib = b_ * 2 + tb
                    nc.scalar.dma_start(x2_d[ib * P:(ib + 1) * P, :], x2s[tb])
                    xn2 = pO.tile([P, D], BF16, tag="xn2", bufs=2, name="xn2")
                    _ln_apply(nc, pO, consts["eps"], x2s[tb], xn2)
                    # transpose xn2 on the PE straight into the fp8 pack (no
                    # DRAM bounce); 4 chunks share one bf16 psum bank so the
                    # fp8 cast runs as one wide DVE op per bank
                    for c4 in range(4):
                        tp2 = psO.tile([P, 4, P], BF16, tag="tp2", bufs=2,
                                       name="tp2")
                        for sub in range(4):
                            cc = c4 * 4 + sub
                            nc.tensor.transpose(
                                tp2[:, sub, :], xn2[:, cc * P:(cc + 1) * P],
                                ident)
                        nc.vector.tensor_copy(
                            out=xn2q[:, c4 * 4:(c4 + 1) * 4,
                                     ib * P:(ib + 1) * P], in_=tp2)
    return xn2q


def _emit_p5(nc, tc, gl, consts, dram_io, dbufs, it, xn2q):
    """MLP over own 512 rows: FF1+GELU (fp8 DoubleRow) -> FF2 + residual."""
    wff1_d, wff2_d, out_d = (dram_io["wff1"], dram_io["wff2"], dram_io["out"])
    x2_d = dbufs["x2_d"]
    bff1, bff2_bc = consts["bff1"], consts["bff2_bc"]

    with ExitStack() as c:
        pM = c.enter_context(tc.tile_pool(name=f"pM_{it}", bufs=1))
        psM = c.enter_context(
            tc.tile_pool(name=f"psM_{it}", bufs=1, space="PSUM"))

        hT = pM.tile([P, 64, 512], F8, tag="hT", bufs=1, name="hT")
        for fb in range(64):
            w1 = pM.tile([P, 16, P], F8, tag="w1", bufs=6, name="w1")
            nc.sync.dma_start(
                w1, wff1_d[:, fb * 2048:(fb + 1) * 2048]
                .rearrange("p (c n) -> p c n", c=16))
            ps = psM.tile([P, 512], F32, tag="ff1", bufs=3, name="ff1ps")
            for cc in range(8):
                nc.tensor.matmul(
                    ps, lhsT=w1[:, 2 * cc:2 * cc + 2, :],
                    rhs=xn2q[:, 2 * cc:2 * cc + 2, :],
                    start=(cc == 0), stop=(cc == 7), perf_mode=DR)
            nc.scalar.activation(hT[:, fb, :], ps, AF.Gelu,
                                 bias=bff1[:, fb:fb + 1], scale=1.0 / SW)

        x2pb = []
        for ib in range(4):
            xt = pM.tile([P, D], BF16, tag="x2pb", bufs=4, name=f"x2pb{ib}")
            nc.sync.dma_start(xt, x2_d[ib * P:(ib + 1) * P, :])
            nc.vector.tensor_add(xt, xt, bff2_bc)
            x2pb.append(xt)

        for dq in range(4):
            psums = [psM.tile([P, 512], F32, tag="ff2", bufs=4, name=f"f2{tb}")
                     for tb in range(4)]
            for qtr in range(4):
                w2 = pM.tile([P, 16, 512], F8, tag="w2", bufs=5, name="w2")
                nc.sync.dma_start(
                    w2, wff2_d[:, :].rearrange("p (c n) -> p c n", c=64)
                    [:, qtr * 16:(qtr + 1) * 16, dq * 512:(dq + 1) * 512])
                for cc in range(8):
                    for tb in range(4):
                        nc.tensor.matmul(
                            psums[tb],
                            lhsT=hT[:, qtr * 16 + 2 * cc:
                                    qtr * 16 + 2 * cc + 2,
                                    tb * P:(tb + 1) * P],
                            rhs=w2[:, 2 * cc:2 * cc + 2, :],
                            start=(qtr == 0 and cc == 0),
                            stop=(qtr == 3 and cc == 7), perf_mode=DR)
            for tb in range(4):
                o1 = pM.tile([P, 512], F32, tag="o1", bufs=4, name="o1")
                nc.vector.scalar_tensor_tensor(
                    out=o1, in0=psums[tb], scalar=1.0 / SW2,
                    in1=x2pb[tb][:, dq * 512:(dq + 1) * 512],
                    op0=ALU.mult, op1=ALU.add)
                nc.scalar.dma_start(
                    out_d[tb * P:(tb + 1) * P, dq * 512:(dq + 1) * 512], o1)


def build_program(loop=1, no_cc=False):
    nc = bacc.Bacc("TRN2", target_bir_lowering=False, debug=False,
                   num_devices=N_CORES)

    dram_io = {
        "x_bf": nc.dram_tensor("x_bf", [ROWS, D], BF16, kind="ExternalInput"),
        "wqkv": nc.dram_tensor("wqkv", [P, 16 * 768], F8, kind="ExternalInput"),
        "bqkv": nc.dram_tensor("bqkv", [P, 6], F32, kind="ExternalInput"),
        "cosT": nc.dram_tensor("cosT", [P, S], BF16, kind="ExternalInput"),
        "sinT": nc.dram_tensor("sinT", [P, S], BF16, kind="ExternalInput"),
        "rotT": nc.dram_tensor("rotT", [P, P], BF16, kind="ExternalInput"),
        "maskT": nc.dram_tensor("maskT", [P, P], BF16, kind="ExternalInput"),
        "ident": nc.dram_tensor("ident", [P, P], BF16, kind="ExternalInput"),
        "wout": nc.dram_tensor("wout", [P, 16 * 2048], F8, kind="ExternalInput"),
        "bout": nc.dram_tensor("bout", [D], BF16, kind="ExternalInput"),
        "wff1": nc.dram_tensor("wff1", [P, 64 * 2048], F8, kind="ExternalInput"),
        "bff1": nc.dram_tensor("bff1", [P, 64], F32, kind="ExternalInput"),
        "wff2": nc.dram_tensor("wff2", [P, 64 * 2048], F8, kind="ExternalInput"),
        "bff2": nc.dram_tensor("bff2", [D], BF16, kind="ExternalInput"),
        "out": nc.dram_tensor("out", [ROWS, D], F32, kind="ExternalOutput"),
    }

    with tile.TileContext(nc) as tc:
        with ExitStack() as ctx:
            cpool = ctx.enter_context(tc.tile_pool(name="consts", bufs=1))
            gpool = ctx.enter_context(tc.tile_pool(name="glob", bufs=1))
            dram = ctx.enter_context(
                tc.tile_pool(name="dram", bufs=1, space="DRAM"))

            def cload(name, shape, dt):
                t = cpool.tile(shape, dt, name=name + "_sb")
                nc.sync.dma_start(t, dram_io[name][:, :])
                return t

            ident = cload("ident", [P, P], BF16)
            maskT = cload("maskT", [P, P], BF16)
            rotT = cload("rotT", [P, P], BF16)
            bqkv = cload("bqkv", [P, 6], F32)
            bff1 = cload("bff1", [P, 64], F32)
            cosT = cload("cosT", [P, S], BF16)
            sinT = cload("sinT", [P, S], BF16)
            ones1 = cpool.tile([1, P], BF16, name="ones1")
            nc.vector.memset(ones1, 1.0)
            ones3_full = cpool.tile([P, 2, 16], F8, name="ones3")
            nc.vector.memset(ones3_full, 1.0)
            ones3 = ones3_full[:, :, 0:1]  # j-step 16B for DoubleRow
            eps_t = cpool.tile([P, 1], F32, name="eps_sb")
            nc.vector.memset(eps_t, EPS)
            bout_bc = cpool.tile([P, D], BF16, name="bout_bc")
            nc.gpsimd.dma_start(
                out=bout_bc, in_=dram_io["bout"].ap()[None, :]
                .to_broadcast((P, D)))
            bff2_bc = cpool.tile([P, D], BF16, name="bff2_bc")
            nc.gpsimd.dma_start(
                out=bff2_bc, in_=dram_io["bff2"].ap()[None, :]
                .to_broadcast((P, D)))
            consts = dict(ident=ident, maskT=maskT, rotT=rotT, bqkv=bqkv,
                          bff1=bff1, cosT=cosT, sinT=sinT, ones1=ones1,
                          ones3=ones3, eps=eps_t, bout_bc=bout_bc,
                          bff2_bc=bff2_bc)

            wqkv_sb = gpool.tile([P, 16, 768], F8, name="wqkv_sb")
            nc.sync.dma_start(
                wqkv_sb, dram_io["wqkv"][:, :]
                .rearrange("p (c n) -> p c n", c=16))
            gl = {"pool": gpool, "wqkv_sb": wqkv_sb}

            def mkbufs(it):
                return {
                    "xn_d": dram.tile([ROWS, D], BF16, name=f"xn_d{it}"),
                    "xnT_sh": dram.tile([D, ROWS], F8, name=f"xnTsh{it}"),
                    "xnT_all": dram.tile([N_CORES * D, ROWS], F8,
                                         name=f"xnTall{it}",
                                         addr_space="Shared"),
                    "aT_sh": [dram.tile([2048, 256], F8, name=f"aTsh{it}_{b}")
                              for b in range(B)],
                    "aT_all": [dram.tile([2048, 256], F8,
                                         name=f"aTall{it}_{b}")
                               for b in range(B)],
                    "x2_d": dram.tile([ROWS, D], BF16, name=f"x2d{it}"),
                }

            dbufs = {0: mkbufs(0)}
            _emit_p1(nc, tc, gl, consts, dram_io, dbufs[0], 0, no_cc)
            for it in range(loop):
                xn2q = _emit_p234(nc, tc, gl, consts, dram_io, dbufs[it], it,
                                  no_cc)
                if it + 1 < loop:
                    dbufs[it + 1] = mkbufs(it + 1)
                    _emit_p1(nc, tc, gl, consts, dram_io, dbufs[it + 1],
                             it + 1, no_cc)
                _emit_p5(nc, tc, gl, consts, dram_io, dbufs[it], it, xn2q)
                del dbufs[it]

    nc.compile()
    return nc


# ----------------------------------------------------------------------------
# host side
# ----------------------------------------------------------------------------

def _bf(a):
    return np.asarray(a, np.float32).astype(ml_dtypes.bfloat16)


def _f8(a, scale):
    return np.clip(np.asarray(a, np.float32) * scale, -240.0, 240.0).astype(
        ml_dtypes.float8_e4m3)


def prepare_inputs(x, cos, sin, mask,
                   ln1_g, ln1_b, w_qkv, b_qkv, w_out, b_out,
                   ln2_g, ln2_b, w_ff1, b_ff1, w_ff2, b_ff2):
    """Fold LN params into weights, shard per core, cast to device dtypes."""
    f32 = np.float32
    x2d = np.asarray(x, f32).reshape(NT, D)
    cos2 = np.asarray(cos, f32).reshape(S, DH)
    sin2 = np.asarray(sin, f32).reshape(S, DH)
    w_qkv = np.asarray(w_qkv, f32); b_qkv = np.asarray(b_qkv, f32)
    w_out = np.asarray(w_out, f32); b_out = np.asarray(b_out, f32)
    w_ff1 = np.asarray(w_ff1, f32); b_ff1 = np.asarray(b_ff1, f32)
    w_ff2 = np.asarray(w_ff2, f32); b_ff2 = np.asarray(b_ff2, f32)
    ln1_g = np.asarray(ln1_g, f32); ln1_b = np.asarray(ln1_b, f32)
    ln2_g = np.asarray(ln2_g, f32); ln2_b = np.asarray(ln2_b, f32)

    w_qkv_f = w_qkv * ln1_g[:, None]
    b_qkv_f = b_qkv + ln1_b @ w_qkv
    w_ff1_f = w_ff1 * ln2_g[:, None]
    b_ff1_f = b_ff1 + ln2_b @ w_ff1

    cosT = _bf(cos2.T / SW).copy()
    sinT = _bf(sin2.T / SW).copy()

    R = np.zeros((P, P), f32)
    for dp in range(64):
        R[dp, dp + 64] = -1.0
        R[dp + 64, dp] = 1.0
    rotT = _bf(R.T).copy()
    maskT = _bf(np.triu(np.ones((P, P), f32)))  # keep k <= q (row=k, col=q)
    ident = _bf(np.eye(P, dtype=f32))

    wff1_r = np.ascontiguousarray(
        _f8(w_ff1_f, SW).reshape(16, P, 64, P).transpose(1, 2, 0, 3)
        .reshape(P, 64 * 2048))
    bff1_r = np.ascontiguousarray(b_ff1_f.reshape(64, P).T)
    wff2_r = np.ascontiguousarray(
        _f8(w_ff2, SW2).reshape(64, P, 2048).transpose(1, 0, 2)
        .reshape(P, 64 * 2048))
    wout_r = np.ascontiguousarray(
        _f8(w_out, SW).reshape(16, P, 2048).transpose(1, 0, 2)
        .reshape(P, 16 * 2048))

    in_maps = []
    for c in range(N_CORES):
        h0 = 2 * c
        cols = np.concatenate([
            np.arange(t * D + h * DH, t * D + (h + 1) * DH)
            for t in range(3) for h in (h0, h0 + 1)])
        x_own = np.ascontiguousarray(np.concatenate([
            x2d[c * 256:(c + 1) * 256],
            x2d[S + c * 256: S + (c + 1) * 256]]))
        wq = np.ascontiguousarray(
            _f8(w_qkv_f[:, cols], SW).reshape(16, P, 768)
            .transpose(1, 0, 2).reshape(P, 16 * 768))
        in_maps.append({
            "x_bf": _bf(x_own),
            "wqkv": wq,
            "bqkv": np.ascontiguousarray(
                (b_qkv_f[cols] * SW).reshape(6, P).T),
            "cosT": cosT, "sinT": sinT, "rotT": rotT,
            "maskT": maskT, "ident": ident,
            "wout": wout_r,
            "bout": _bf(b_out),
            "wff1": wff1_r, "bff1": bff1_r,
            "wff2": wff2_r, "bff2": _bf(b_ff2),
        })
    return in_maps


class SpmdExec:
    """Compile once; run the SPMD program on 8 cores without donation so the
    call can be repeated for timing."""

    def __init__(self, nc):
        import jax
        from jax.sharding import Mesh, PartitionSpec
        from jax.experimental.shard_map import shard_map
        from concourse import bass2jax

        bass2jax.install_neuronx_cc_hook()
        self._jax = jax
        self.nc = nc
        pname = nc.partition_id_tensor.name if nc.partition_id_tensor else None
        in_names, out_names, out_avals, zeros = [], [], [], []
        for alloc in nc.m.functions[0].allocations:
            if not isinstance(alloc, mybir.MemoryLocationSet):
                continue
            name = alloc.memorylocations[0].name
            if alloc.kind == "ExternalInput":
                if name != pname:
                    in_names.append(name)
            elif alloc.kind == "ExternalOutput":
                out_names.append(name)
                shape = tuple(alloc.tensor_shape)
                dtype = mybir.dt.np(alloc.dtype)
                out_avals.append(jax.core.ShapedArray(shape, dtype))
                zeros.append(np.zeros(shape, dtype))
        self.in_names, self.out_names = in_names, out_names
        self.out_avals = out_avals
        n_params = len(in_names)
        all_names = in_names + out_names + ([pname] if pname else [])

        def _body(*args):
            ops = list(args)
            if pname:
                ops.append(bass2jax.partition_id_tensor())
            outs = bass2jax._bass_exec_p.bind(
                *ops, out_avals=tuple(out_avals), in_names=tuple(all_names),
                out_names=tuple(out_names), lowering_input_output_aliases=(),
                sim_require_finite=False, sim_require_nnan=False, nc=nc)
            return tuple(outs)

        devices = jax.devices()[:N_CORES]
        mesh = Mesh(np.asarray(devices), ("core",))
        in_specs = (PartitionSpec("core"),) * (n_params + len(out_names))
        out_specs = (PartitionSpec("core"),) * len(out_names)
        self.fn = jax.jit(
            shard_map(_body, mesh=mesh, in_specs=in_specs, out_specs=out_specs,
                      check_rep=False),
            keep_unused=True)
        self._zeros = zeros
        self._dev_args = None

    def place(self, in_maps):
        jax = self._jax
        from jax.sharding import Mesh, PartitionSpec, NamedSharding
        devices = jax.devices()[:N_CORES]
        mesh = Mesh(np.asarray(devices), ("core",))
        concat_in = [np.concatenate([np.asarray(in_maps[c][n])
                                     for c in range(N_CORES)], axis=0)
                     for n in self.in_names]
        concat_zero = [np.zeros((N_CORES * z.shape[0], *z.shape[1:]), z.dtype)
                       for z in self._zeros]
        self._dev_args = [
            jax.device_put(a, NamedSharding(mesh, PartitionSpec("core")))
            for a in concat_in + concat_zero]

    def run_raw(self):
        return self._jax.block_until_ready(self.fn(*self._dev_args))

    def run(self):
        jax = self._jax
        outs = jax.block_until_ready(self.fn(*self._dev_args))
        res = []
        for c in range(N_CORES):
            res.append({
                name: np.asarray(outs[i]).reshape(
                    N_CORES, *self.out_avals[i].shape)[c]
                for i, name in enumerate(self.out_names)})
        return res


def get_exec(loop=1, no_cc=False):
    key = (loop, no_cc)
    if key not in _EXEC_CACHE:
        nc = build_program(loop, no_cc=no_cc)
        _EXEC_CACHE[key] = SpmdExec(nc)
    return _EXEC_CACHE[key]


def assemble_output(res):
    out = np.zeros((NT, D), np.float32)
    for c in range(N_CORES):
        o = res[c]["out"]
        out[c * 256:(c + 1) * 256] = o[:256]
        out[S + c * 256: S + (c + 1) * 256] = o[256:]
    return out.reshape(B, S, D)


def kernel(**inputs):
    ex = get_exec(loop=1)
    in_maps = prepare_inputs(**inputs)
    ex.place(in_maps)
    res = ex.run()
    return assemble_output(res).astype(np.float32)
